# revision 9
# baseline (speedup 1.0000x reference)
"""Trainium2 Bass kernel for nn_NetSoNTopSIAMReg (adaptive-avg-pool + per-sample
top-k vote prefix sums).

Reference computation:
    x_sun = mean(maps, axis=(2,3))                        # [B, A]
    vote  = x_sun[:, None, :] * weight                    # [B, 1, A]
    sort |vote| desc; csum = cumsum(sorted_vote)
    x_topk[k] = csum[k-1] + avg   (k = 1..8)
    x_dense   = sum(vote) + avg
    x_son = [x_topk(1..8), x_dense]                       # [9, B, 1]

Sharding: data-parallel over batch B=32 across 8 cores (4 samples/core);
weight and avg_value replicated.  Each core streams its 4x102x224x224 f32
shard (81.9 MB) from HBM, reducing spatial dims on DVE + ACT in parallel,
then runs the tiny top-8 selection loop on-chip.
"""

import numpy as np

import concourse.bass as bass
import concourse.bacc as bacc
import concourse.mybir as mybir
from concourse import tile
from concourse.bass_utils import run_bass_kernel_spmd

B, A, H, W = 32, 102, 224, 224
S = H * W              # 50176 spatial elements per (b, a)
M = 8                  # cores
BS = B // M            # 4 samples per core
NCH = 8                # spatial chunks per sample
F = S // NCH           # 6272 elements per chunk
TOPK = 8
NOUT = TOPK + 1        # 8 top-k prefix sums + 1 dense sum
INV_S = 1.0 / S
BIG = 1.0e30
FP = mybir.dt.float32
AX = mybir.AxisListType
ALU = mybir.AluOpType
ACTF = mybir.ActivationFunctionType


def build_program() -> bass.Bass:
    nc = bacc.Bacc("TRN2", debug=False)

    maps_in = nc.dram_tensor("maps", [BS, A, S], FP, kind="ExternalInput")
    w_in = nc.dram_tensor("weight", [1, A], FP, kind="ExternalInput")
    avg_in = nc.dram_tensor("avg", [1, 1], FP, kind="ExternalInput")
    sun_out = nc.dram_tensor("x_sun", [BS, A], FP, kind="ExternalOutput")
    son_out = nc.dram_tensor("x_son", [BS, NOUT], FP, kind="ExternalOutput")

    with tile.TileContext(nc) as tc:
        with (
            tc.tile_pool(name="inp", bufs=4) as inp_pool,
            tc.tile_pool(name="dump", bufs=1) as dump_pool,
            tc.tile_pool(name="acc", bufs=2) as acc_pool,
            tc.tile_pool(name="small", bufs=8) as small_pool,
            tc.tile_pool(name="p2", bufs=1) as p2_pool,
        ):
            # x_sun gathered with samples on partitions, channels on free dim.
            xsun_t = p2_pool.tile([BS, A], FP, tag="xsun_t")

            # ---- Phase 1: spatial mean per (sample, channel) ----
            for s in range(BS):
                acc = acc_pool.tile([A, NCH], FP, tag="acc")
                for j in range(NCH):
                    t = inp_pool.tile([A, F], FP, tag="inp")
                    # SWDGE: HWDGE direct2d codegen rejects >1 sync-wait per DMA,
                    # and slot-reuse DMAs carry two (consumer sem + prior-DMA sem).
                    nc.gpsimd.dma_start(out=t[:, :], in_=maps_in[s, :, j * F:(j + 1) * F])
                    if j % 2 == 0:
                        nc.vector.reduce_sum(out=acc[:, j:j + 1], in_=t[:, :], axis=AX.X)
                    else:
                        d = dump_pool.tile([A, F], FP, tag="dump")
                        nc.scalar.activation(
                            out=d[:, :], in_=t[:, :], func=ACTF.Copy,
                            accum_out=acc[:, j:j + 1],
                        )
                sums = small_pool.tile([A, 1], FP, tag="sums")
                nc.vector.reduce_sum(out=sums[:, :], in_=acc[:, :], axis=AX.X)
                xs = small_pool.tile([A, 1], FP, tag="xs")
                nc.scalar.activation(out=xs[:, :], in_=sums[:, :], func=ACTF.Copy, scale=INV_S)
                # partition-major [A,1] -> one free-dim row of xsun_t (SBUF->SBUF)
                nc.gpsimd.dma_start(out=xsun_t[s:s + 1, :], in_=xs[:, 0:1])

            # ---- Phase 2: votes + top-8 prefix sums (tiny) ----
            wt = p2_pool.tile([BS, A], FP, tag="wt")
            avt = p2_pool.tile([BS, 1], FP, tag="avt")
            for b in range(BS):
                nc.gpsimd.dma_start(out=wt[b:b + 1, :], in_=w_in[0:1, :])
                nc.gpsimd.dma_start(out=avt[b:b + 1, 0:1], in_=avg_in[0:1, 0:1])

            vote = p2_pool.tile([BS, A], FP, tag="vote")
            nc.vector.tensor_tensor(out=vote[:, :], in0=xsun_t[:, :], in1=wt[:, :], op=ALU.mult)
            absv = p2_pool.tile([BS, A], FP, tag="absv")
            nc.scalar.activation(out=absv[:, :], in_=vote[:, :], func=ACTF.Abs)

            # NOTE: tensor_tensor_reduce passes CoreSim + walrus but crashes the
            # device (NRT unrecoverable) — avoid it; use mul + reduce + add.
            csums = p2_pool.tile([BS, NOUT], FP, tag="csums")
            # dense pass: csums[:, 8] = avg + sum(vote)
            tot = small_pool.tile([BS, 1], FP, tag="tot")
            nc.vector.reduce_sum(out=tot[:, :], in_=vote[:, :], axis=AX.X)
            nc.vector.tensor_scalar(
                out=csums[:, TOPK:TOPK + 1], in0=tot[:, :], scalar1=avt[:, 0:1],
                scalar2=None, op0=ALU.add,
            )
            prev = avt[:, 0:1]  # running prefix sum, seeded with avg
            for k in range(TOPK):
                m = small_pool.tile([BS, 1], FP, tag="m")
                nc.vector.reduce_max(out=m[:, :], in_=absv[:, :], axis=AX.X)
                mask = small_pool.tile([BS, A], FP, tag="mask")
                nc.vector.tensor_scalar(
                    out=mask[:, :], in0=absv[:, :], scalar1=m[:, 0:1], scalar2=None,
                    op0=ALU.is_ge,
                )
                sel = small_pool.tile([BS, A], FP, tag="sel")
                nc.vector.tensor_tensor(out=sel[:, :], in0=vote[:, :], in1=mask[:, :], op=ALU.mult)
                step = small_pool.tile([BS, 1], FP, tag="step")
                nc.vector.reduce_sum(out=step[:, :], in_=sel[:, :], axis=AX.X)
                # csums[:, k] = prev + step
                nc.vector.tensor_tensor(
                    out=csums[:, k:k + 1], in0=step[:, :], in1=prev, op=ALU.add,
                )
                prev = csums[:, k:k + 1]
                if k < TOPK - 1:
                    bigm = small_pool.tile([BS, A], FP, tag="bigm")
                    nc.vector.tensor_scalar_mul(out=bigm[:, :], in0=mask[:, :], scalar1=BIG)
                    nc.vector.tensor_sub(out=absv[:, :], in0=absv[:, :], in1=bigm[:, :])

            nc.gpsimd.dma_start(out=sun_out[:, :], in_=xsun_t[:, :])
            nc.gpsimd.dma_start(out=son_out[:, :], in_=csums[:, :])

    nc.compile()
    return nc


def _install_axon_ntff_shim():
    """bass_utils' trace=True path under axon imports ``antenv.axon_hooks``,
    which this image lacks; synthesize the module so NTFF profiling works.
    Degrades to trace-disabled on any failure."""
    import sys
    import types

    if "antenv.axon_hooks" in sys.modules:
        return
    try:
        from trn_agent_boot.trn_boot import _ntff_profile_via_ctypes

        hook = _ntff_profile_via_ctypes("/opt/axon/libaxon_pjrt.so")
        mod = types.ModuleType("antenv.axon_hooks")
        mod._hook = hook
        mod.get_axon_ntff_profile_hook = lambda: mod._hook

        def _set(h):
            mod._hook = h

        mod.set_axon_ntff_profile_hook = _set
        sys.modules["antenv.axon_hooks"] = mod
    except Exception:
        import os

        os.environ.setdefault("BASS_NEVER_TRACE", "1")


_NC_CACHE: list = []
LAST_RESULTS = None  # BassKernelResults of the most recent kernel() call


def _get_nc() -> bass.Bass:
    if not _NC_CACHE:
        _NC_CACHE.append(build_program())
    return _NC_CACHE[0]


def kernel(maps, weight, avg_value):
    maps = np.ascontiguousarray(np.asarray(maps, dtype=np.float32)).reshape(B, A, S)
    weight = np.ascontiguousarray(np.asarray(weight, dtype=np.float32)).reshape(1, A)
    avg = np.asarray(avg_value, dtype=np.float32).reshape(1, 1)

    _install_axon_ntff_shim()
    nc = _get_nc()
    in_maps = [
        {"maps": maps[i * BS:(i + 1) * BS], "weight": weight, "avg": avg}
        for i in range(M)
    ]
    res = run_bass_kernel_spmd(nc, in_maps, core_ids=list(range(M)))
    global LAST_RESULTS
    LAST_RESULTS = res
    outs = res.results

    x_sun = np.concatenate([outs[i]["x_sun"] for i in range(M)], axis=0)
    son = np.concatenate([outs[i]["x_son"] for i in range(M)], axis=0)   # [B, 9]
    x_son = np.ascontiguousarray(son.T)[:, :, None]                      # [9, B, 1]
    return x_sun.astype(np.float32), x_son.astype(np.float32)
